# revision 34
# baseline (speedup 1.0000x reference)
"""Bilateral filter (7x7, dilation 1) Trainium2 Bass kernel — v7.

Problem: input [2, 18, 1024, 1024] f32.
  filterable = input[:, :8]; params = input[:, 8:]
  r_c = -(p_c^2), sx = -(p8^2), sy = -(p9^2)
  logw = sum_c r_c (fn_c - f_c)^2 + sx dx^2 + sy dy^2  (OOB taps masked)
  out[c] = sum_taps w * fn_c / sum_taps w,  c < 3

Sharding: data-parallel over (batch, H): 8 cores, each 256 rows of one batch
image (+3 halo rows each side, sentinel-padded host-side, sentinel=8).

Design (per core, 2 row-blocks x 2 W-chunks of [128 rows, 512 cols]):
  - fp16 on-chip compute; channel-planar free-axis layout [128, 8ch*518col]
    keeps every hot access-pattern unit-stride => DVE 2x_1P mode throughout.
  - Hot loop runs on DVE+ACT only.  GPSIMD tensor ops share the DVE SBUF
    port and throttle DVE ~1.8x while active (measured), so GPSIMD only
    does cast-DMA descriptor generation and memsets.
  - Per tap: DVE sub -> ACT Square (in-place) -> DVE m=p^2*q ->
    DVE pairwise-tree channel reduce -> +Asp -> ACT exp(scale=-1) ->
    DVE w*fn3 -> PE identity-matmul accumulates [w*fn3 | w] into PSUM
    (fp32) across all 49 taps.  4-stage software pipeline so every
    cross-engine dependency is issued >=1 full tap ahead.
  - Macro-boundary overlap: staging tiles are double-buffered and loaded
    (GPSIMD cast-DMA f32->f16) during the previous macro's taps; the 12
    partition-shifted SBUF->SBUF copies that build the 7 row-shifted tile
    sets are issued as soon as the tap loop stops reading each F tile.
  - Weights: w = exp(-(sum_c p_c^2 d_c^2 + a*sx^2 + b*sy^2)) with all
    terms positive; sentinel pixels drive the sum to ~1e4 so exp -> +0,
    reproducing the reference's OOB mask (no inf anywhere: max sum ~41K
    < fp16 max 65504).
"""

import sys

if "/opt/trn_rl_repo" not in sys.path:
    sys.path.insert(0, "/opt/trn_rl_repo")

import numpy as np

import concourse.bass as bass
import concourse.mybir as mybir
from concourse.bacc import Bacc
from concourse.tile import TileContext
from concourse.masks import make_identity

FP32 = mybir.dt.float32
FP16 = mybir.dt.float16
AF = mybir.ActivationFunctionType

B, C_ALL, H, W = 2, 18, 1024, 1024
CF = 8                      # filterable channels
CO = 3                      # output channels
KS, RAD = 7, 3
HC = H * B // 8             # 256 output rows per core
HIN = HC + 2 * RAD          # 262 input rows per core (halo padded host-side)
WC = 512                    # W chunk
NW = W // WC                # 2
NHB = HC // 128             # 2
WT = WC + 2 * RAD           # 518 (with column halo)
SENT = 8.0                  # sentinel: max quadratic form ~41K < fp16 max,
                            # so no inf on-chip, yet exp(-s) underflows to +0
D2IDX = [3, 2, 1, 0, 1, 2, 3]              # index into D2VALS: (k-3)^2
D2VALS = [0.0, 1.0, 4.0, 9.0]
CTR = KS // 2 * KS + KS // 2               # center tap (w == 1 fast path)

_CACHED = {}


def _cm(ap, w=WT, c=CF):
    """View flat [128, c*w] as [128, c, w] (channel-major blocks)."""
    return ap.rearrange("p (c x) -> p c x", c=c)


def build_nc():
    nc = Bacc()
    x = nc.dram_tensor("x", [HIN, C_ALL, W], FP32, kind="ExternalInput")
    y = nc.dram_tensor("y", [CO, HC, W], FP32, kind="ExternalOutput")

    macros = [(hb, wck) for hb in range(NHB) for wck in range(NW)]
    NM = len(macros)

    with TileContext(nc) as tc:
        with (
            tc.tile_pool(name="ipool", bufs=1) as ipool,
            tc.tile_pool(name="fpool", bufs=1) as fpool,
            tc.tile_pool(name="cpool", bufs=1) as cpool,
            tc.tile_pool(name="dpool", bufs=1) as dpool,
            tc.tile_pool(name="spool", bufs=1) as spool,
            tc.tile_pool(name="ppool", bufs=1, space="PSUM") as ppool,
        ):
            ident = ipool.tile([128, 128], FP16, tag="ident", name="ident")
            make_identity(nc, ident[:])

            Fk = {}     # macro idx -> {oy: tile}

            def make_ftile(k, oy):
                """F[oy][p] = slab row r0+oy+p, cast f32->f16 straight from
                DRAM (write-only SBUF traffic: no staging, no SBUF->SBUF
                copies contending with DVE reads)."""
                hb, wck = macros[k]
                w0, r0 = wck * WC, hb * 128
                lo = RAD if wck == 0 else 0
                hi = WT - RAD if wck == NW - 1 else WT
                Ft = fpool.tile([128, CF * WT], FP16, tag=f"F{oy}",
                                bufs=2 if oy == RAD else 1,
                                name=f"F{oy}_{k}")
                v = Ft[:].rearrange("p (c x) -> p c x", c=CF)
                if lo > 0:
                    nc.gpsimd.memset(v[:, :, 0:lo], SENT)
                if hi < WT:
                    nc.gpsimd.memset(v[:, :, hi:WT], SENT)
                nc.gpsimd.dma_start(
                    out=v[:, :, lo:hi],
                    in_=x[r0 + oy : r0 + oy + 128, 0:CF,
                          w0 - RAD + lo : w0 - RAD + hi],
                )
                Fk.setdefault(k, {})[oy] = Ft

            # Load order = tap consumption order: the first groups touch
            # F[0] and the center tile F[3]; the rest can stream in while
            # the first row's taps already run (the 7 transfers are ~43us
            # of DMA bandwidth, so ordering decides when compute starts).
            for oy in (0, RAD, 1, 2, 4, 5, 6):
                make_ftile(0, oy)

            for k in range(NM):
                _macro(nc, tc, x, y, ident, fpool, cpool, dpool, spool,
                       ppool, macros, k, Fk, make_ftile)
    nc.compile()
    return nc


def _macro(nc, tc, x, y, ident, fpool, cpool, dpool, spool, ppool,
           macros, k, Fk, make_ftile):
    hb, wck = macros[k]
    NM = len(macros)
    w0 = wck * WC
    r0 = hb * 128
    F = Fk[k]
    Fc = _cm(F[RAD][:])[:, :, RAD : RAD + WC]

    # ---- params: P2[c] = p_c^2 (f16, c-major), sx2/sy2 ----
    # DMAs ride the ACT queue (HWDGE) so they never serialize behind the
    # F-tile shift copies on the sync queue.
    P2 = cpool.tile([128, CF * WC], FP16, tag="P2", name=f"P2_{k}")
    sxy2 = cpool.tile([128, 2 * WC], FP16, tag="sxy2", name=f"sxy2_{k}")
    for kk in range(CF + 2):
        pst = fpool.tile([128, WC], FP32, tag="pst", bufs=2,
                         name=f"pst_{k}_{kk}")
        nc.scalar.dma_start(
            out=pst[:],
            in_=x[r0 + RAD : r0 + RAD + 128, CF + kk, w0 : w0 + WC])
        dst = (P2[:, kk * WC : (kk + 1) * WC] if kk < CF
               else sxy2[:, (kk - CF) * WC : (kk - CF + 1) * WC])
        nc.scalar.activation(dst, pst[:], AF.Square)
    sx2 = sxy2[:, 0:WC]
    sy2 = sxy2[:, WC : 2 * WC]

    # ---- spatial log-weights, one tile of 16 slots: slot(bi*4+ai) =
    # D2VALS[ai]*sx2 + D2VALS[bi]*sy2.  Slot order is chosen so that the
    # two taps of every paired group land in adjacent (or equal) slots,
    # making the paired asp-add a plain contiguous slice. ----
    Asp16 = cpool.tile([128, 16 * WC], FP16, tag="Asp16", name=f"A16_{k}")
    for ai in (1, 2, 3):                       # bi = 0 row
        nc.vector.tensor_scalar_mul(
            Asp16[:, ai * WC : (ai + 1) * WC], sx2, float(D2VALS[ai]))
    for bi in (1, 2, 3):                       # ai = 0 column
        nc.vector.tensor_scalar_mul(
            Asp16[:, bi * 4 * WC : (bi * 4 + 1) * WC], sy2,
            float(D2VALS[bi]))
    for ai in (1, 2, 3):
        for bi in (1, 2, 3):
            s = bi * 4 + ai
            nc.vector.tensor_add(
                Asp16[:, s * WC : (s + 1) * WC],
                Asp16[:, bi * 4 * WC : (bi * 4 + 1) * WC],
                Asp16[:, ai * WC : (ai + 1) * WC])
    A3 = Asp16[:].rearrange("p (s x) -> p s x", s=16)

    def asp_slot(i, j):
        return D2IDX[i] * 4 + D2IDX[j]

    # ---- PSUM accumulator: [w*fn0 | w*fn1 | w*fn2 | w] ----
    ps = ppool.tile([128, 4 * WC], FP32, tag="ps", bufs=2, name=f"ps_{k}")

    # ---- tap groups: pairs of taps share one d-tile and run the square,
    # p^2-multiply, tree, asp and exp as single double-width ops, halving
    # the ~150ns fixed cost per DVE op.  Within a pair the taps are ordered
    # by asp slot so the paired asp-add reads adjacent slots. ----
    groups = []          # (i, [j...]) with len 1 or 2; None marks center
    row_last = {}
    for i in range(KS):
        gl = ([[0, 1], [2, 3], [4, 5], [6]] if i != RAD
              else [[0, 1], [2, 4], [5, 6], None])
        for js in gl:
            if js is None:
                groups.append((i, None))
            else:
                groups.append((i, sorted(js, key=lambda j: asp_slot(i, j))))
        row_last[i] = len(groups) - 1
    n = len(groups)
    DW = 2 * CF * WC     # d-tile width (two taps)
    TW = 2 * 4 * WC
    Dt, Tt = {}, {}

    def gv(ap, G, w):    # [128, G*w] -> [128, G, w]
        return ap.rearrange("p (g x) -> p g x", g=G)

    def stage_sub(g):    # per-tap subs into the halves of one shared tile
        i, js = groups[g]
        if js is None:
            return
        d = dpool.tile([128, DW], FP16, tag="d", bufs=4, name=f"d_{k}_{g}")
        for gi, j in enumerate(js):
            nc.vector.tensor_sub(
                _cm(d[:, gi * CF * WC : (gi + 1) * CF * WC], WC),
                _cm(F[i][:])[:, :, j : j + WC], Fc)
        Dt[g] = d

    def stage_sq(g):    # one ACT square over both halves, in-place
        i, js = groups[g]
        if js is None:
            return
        d = Dt[g]
        nc.scalar.activation(d[:, 0 : len(js) * CF * WC],
                             d[:, 0 : len(js) * CF * WC], AF.Square)

    def stage_m(g):     # m = p^2 * q over both halves (P2 broadcast)
        i, js = groups[g]
        if js is None:
            return
        G = len(js)
        dv = gv(Dt[g][:, 0 : G * CF * WC], G, CF * WC)
        p2b = P2[:].unsqueeze(1).broadcast_to([128, G, CF * WC])
        nc.vector.tensor_mul(dv, p2b, dv)

    def tree(g, lvl):   # halve each tap's channel block, both taps at once

        i, js = groups[g]
        if js is None:
            return
        G = len(js)
        hw = (CF >> (lvl - 1)) * WC       # block width entering this level
        dv = gv(Dt[g][:, 0 : G * CF * WC], G, CF * WC)
        nc.vector.tensor_add(dv[:, :, 0 : hw // 2], dv[:, :, 0 : hw // 2],
                             dv[:, :, hw // 2 : hw])

    def stage_we(g):    # +asp (paired slot read), exp -> w
        i, js = groups[g]
        T = spool.tile([128, TW], FP16, tag="T", bufs=3, name=f"T_{k}_{g}")
        Tt[g] = T
        if js is None:
            nc.gpsimd.memset(T[:, 3 * WC : 4 * WC], 1.0)
            return
        G = len(js)
        dv = gv(Dt.pop(g)[:, 0 : G * CF * WC], G, CF * WC)[:, :, 0:WC]
        s0 = asp_slot(i, js[0])
        if G == 2:
            s1 = asp_slot(i, js[1])
            aspv = (A3[:, s0 : s0 + 2, :] if s1 == s0 + 1
                    else A3[:, s0 : s0 + 1, :].broadcast_to([128, 2, WC]))
        else:
            aspv = A3[:, s0 : s0 + 1, :]
        nc.vector.tensor_add(dv, dv, aspv)
        tv = gv(T[:, 0 : G * 4 * WC], G, 4 * WC)
        nc.scalar.activation(tv[:, :, 3 * WC : 4 * WC], dv, AF.Exp,
                             scale=-1.0)

    def stage_c(g):     # w*fn3 per tap, PE accumulates [w*fn3 | w] chunks
        i, js = groups[g]
        T = Tt.pop(g)
        if js is None:
            fn3 = _cm(F[i][:])[:, 0:CO, RAD : RAD + WC]
            nc.vector.tensor_copy(_cm(T[:, 0 : CO * WC], WC, CO), fn3)
            js_eff = [RAD]
        else:
            js_eff = js
            for gi, j in enumerate(js):
                o = gi * 4 * WC
                w_b = T[:, o + 3 * WC : o + 4 * WC].unsqueeze(1).broadcast_to(
                    [128, CO, WC])
                fn3 = _cm(F[i][:])[:, 0:CO, j : j + WC]
                nc.vector.tensor_mul(
                    _cm(T[:, o : o + CO * WC], WC, CO), w_b, fn3)
        for gi in range(len(js_eff)):
            for kk in range(4):
                nc.tensor.matmul(
                    ps[:, kk * WC : (kk + 1) * WC], ident[:],
                    T[:, (gi * 4 + kk) * WC : (gi * 4 + kk + 1) * WC],
                    start=(g == 0 and gi == 0),
                    stop=(g == n - 1 and gi == len(js_eff) - 1))

    # DVE issue order interleaves the dependent tree chain of group g-2 with
    # independent ops (sub of g, m of g-1, mul3 of g-3) so each op's pipe
    # DRAIN overlaps an unrelated op instead of stalling its consumer.
    # ACT order per iteration: exp(g-2) before square(g), so the exp->mul3
    # chain never queues behind the big square.
    for g in range(n + 3):
        if 0 <= g - 2 < n:
            tree(g - 2, 1)
        if g < n:
            stage_sub(g)
        if 0 <= g - 2 < n:
            tree(g - 2, 2)
        if 0 <= g - 1 < n:
            stage_m(g - 1)
        if 0 <= g - 2 < n:
            tree(g - 2, 3)
        if 0 <= g - 3 < n:
            stage_c(g - 3)
        if 0 <= g - 2 < n:
            stage_we(g - 2)
        if g < n:
            stage_sq(g)
        # Row block i of F is last read by stage_c(row_last[i]), issued at
        # iteration row_last[i]+3: rebuild it for the next macro after that.
        if k + 1 < NM and 0 <= g - 3 < n:
            i_done, js_done = groups[g - 3]
            if g - 3 == row_last[i_done]:
                make_ftile(k + 1, i_done)

    # ---- out = acc / wsum ----
    rec = spool.tile([128, WC], FP32, tag="rec", name=f"rec_{k}")
    nc.vector.reciprocal(rec[:], ps[:, 3 * WC : 4 * WC])
    out3 = spool.tile([128, CO * WC], FP32, tag="out3", name=f"o3_{k}")
    rec_b = rec[:].unsqueeze(1).broadcast_to([128, CO, WC])
    nc.vector.tensor_mul(_cm(out3[:], WC, CO), rec_b,
                         _cm(ps[:, 0 : CO * WC], WC, CO))
    for c in range(CO):
        nc.sync.dma_start(out=y[c, r0 : r0 + 128, w0 : w0 + WC],
                          in_=out3[:, c * WC : (c + 1) * WC])


def shard_inputs(input):
    """input [2,18,1024,1024] -> 8 per-core slabs [262, 18, 1024]."""
    input = np.asarray(input, dtype=np.float32)
    per_b = 4
    rows = H // per_b
    in_maps = []
    for core in range(8):
        b, q = divmod(core, per_b)
        r0 = q * rows
        slab = np.full((HIN, C_ALL, W), SENT, dtype=np.float32)
        s_lo = max(r0 - RAD, 0)
        s_hi = min(r0 + rows + RAD, H)
        slab[s_lo - (r0 - RAD) : s_hi - (r0 - RAD), :, :] = (
            input[b, :, s_lo:s_hi, :].transpose(1, 0, 2))
        in_maps.append({"x": np.ascontiguousarray(slab)})
    return in_maps


def assemble(results):
    out = np.empty((B, CO, H, W), dtype=np.float32)
    rows = H // 4
    for core in range(8):
        b, q = divmod(core, 4)
        out[b, :, q * rows : (q + 1) * rows, :] = results[core]["y"]
    return out


def kernel(input):
    from concourse.bass_utils import run_bass_kernel_spmd

    if "nc" not in _CACHED:
        _CACHED["nc"] = build_nc()
    in_maps = shard_inputs(input)
    res = run_bass_kernel_spmd(_CACHED["nc"], in_maps, list(range(8)))
    return assemble(res.results)


# revision 40
# speedup vs baseline: 1.2078x; 1.2078x over previous
"""Bilateral filter (7x7, dilation 1) Trainium2 Bass kernel — v7.

Problem: input [2, 18, 1024, 1024] f32.
  filterable = input[:, :8]; params = input[:, 8:]
  r_c = -(p_c^2), sx = -(p8^2), sy = -(p9^2)
  logw = sum_c r_c (fn_c - f_c)^2 + sx dx^2 + sy dy^2  (OOB taps masked)
  out[c] = sum_taps w * fn_c / sum_taps w,  c < 3

Sharding: data-parallel over (batch, H): 8 cores, each 256 rows of one batch
image (+3 halo rows each side, sentinel-padded host-side, sentinel=8).

Design (per core, 2 row-blocks x 2 W-chunks of [128 rows, 512 cols]):
  - fp16 on-chip compute; channel-planar free-axis layout [128, 8ch*518col]
    keeps every hot access-pattern unit-stride => DVE 2x_1P mode throughout.
  - Hot loop runs on DVE+ACT only.  GPSIMD tensor ops share the DVE SBUF
    port and throttle DVE ~1.8x while active (measured), so GPSIMD only
    does cast-DMA descriptor generation and memsets.
  - Per tap: DVE sub -> ACT Square (in-place) -> DVE m=p^2*q ->
    DVE pairwise-tree channel reduce -> +Asp -> ACT exp(scale=-1) ->
    DVE w*fn3 -> PE identity-matmul accumulates [w*fn3 | w] into PSUM
    (fp32) across all 49 taps.  4-stage software pipeline so every
    cross-engine dependency is issued >=1 full tap ahead.
  - Macro-boundary overlap: staging tiles are double-buffered and loaded
    (GPSIMD cast-DMA f32->f16) during the previous macro's taps; the 12
    partition-shifted SBUF->SBUF copies that build the 7 row-shifted tile
    sets are issued as soon as the tap loop stops reading each F tile.
  - Weights: w = exp(-(sum_c p_c^2 d_c^2 + a*sx^2 + b*sy^2)) with all
    terms positive; sentinel pixels drive the sum to ~1e4 so exp -> +0,
    reproducing the reference's OOB mask (no inf anywhere: max sum ~41K
    < fp16 max 65504).
"""

import sys

if "/opt/trn_rl_repo" not in sys.path:
    sys.path.insert(0, "/opt/trn_rl_repo")

import numpy as np

import concourse.bass as bass
import concourse.mybir as mybir
from concourse.bacc import Bacc
from concourse.tile import TileContext
from concourse.masks import make_identity

FP32 = mybir.dt.float32
FP16 = mybir.dt.float16
AF = mybir.ActivationFunctionType

B, C_ALL, H, W = 2, 18, 1024, 1024
CF = 8                      # filterable channels
CO = 3                      # output channels
KS, RAD = 7, 3
HC = H * B // 8             # 256 output rows per core
HIN = HC + 2 * RAD          # 262 input rows per core (halo padded host-side)
WC = 512                    # W chunk
NW = W // WC                # 2
NHB = HC // 128             # 2
WT = WC + 2 * RAD           # 518 (with column halo)
SENT = 8.0                  # sentinel: max quadratic form ~41K < fp16 max,
                            # so no inf on-chip, yet exp(-s) underflows to +0
D2IDX = [3, 2, 1, 0, 1, 2, 3]              # index into D2VALS: (k-3)^2
D2VALS = [0.0, 1.0, 4.0, 9.0]
CTR = KS // 2 * KS + KS // 2               # center tap (w == 1 fast path)

_CACHED = {}


def _cm(ap, w=WT, c=CF):
    """View flat [128, c*w] as [128, c, w] (channel-major blocks)."""
    return ap.rearrange("p (c x) -> p c x", c=c)


def build_nc():
    nc = Bacc()
    x = nc.dram_tensor("x", [HIN, C_ALL, W], FP32, kind="ExternalInput")
    # y carries [acc0, acc1, acc2, wsum]; the division happens on the host
    # (DVE reciprocal is an iterative ~8cyc/elem op — not worth DVE time)
    y = nc.dram_tensor("y", [CO + 1, HC, W], FP32, kind="ExternalOutput")

    macros = [(hb, wck) for hb in range(NHB) for wck in range(NW)]
    NM = len(macros)

    with TileContext(nc) as tc:
        with (
            tc.tile_pool(name="ipool", bufs=1) as ipool,
            tc.tile_pool(name="fpool", bufs=1) as fpool,
            tc.tile_pool(name="cpool", bufs=1) as cpool,
            tc.tile_pool(name="dpool", bufs=1) as dpool,
            tc.tile_pool(name="spool", bufs=1) as spool,
            tc.tile_pool(name="ppool", bufs=1, space="PSUM") as ppool,
        ):
            ident = ipool.tile([128, 128], FP16, tag="ident", name="ident")
            make_identity(nc, ident[:])

            Fk = {}     # macro idx -> {oy: tile}

            def make_ftile(k, oy):
                """F[oy][p] = slab row r0+oy+p, cast f32->f16 straight from
                DRAM (write-only SBUF traffic: no staging, no SBUF->SBUF
                copies contending with DVE reads)."""
                hb, wck = macros[k]
                w0, r0 = wck * WC, hb * 128
                lo = RAD if wck == 0 else 0
                hi = WT - RAD if wck == NW - 1 else WT
                Ft = fpool.tile([128, CF * WT], FP16, tag=f"F{oy}",
                                bufs=2 if oy == RAD else 1,
                                name=f"F{oy}_{k}")
                v = Ft[:].rearrange("p (c x) -> p c x", c=CF)
                if lo > 0:
                    nc.gpsimd.memset(v[:, :, 0:lo], SENT)
                if hi < WT:
                    nc.gpsimd.memset(v[:, :, hi:WT], SENT)
                nc.gpsimd.dma_start(
                    out=v[:, :, lo:hi],
                    in_=x[r0 + oy : r0 + oy + 128, 0:CF,
                          w0 - RAD + lo : w0 - RAD + hi],
                )
                Fk.setdefault(k, {})[oy] = Ft

            # Load order = tap consumption order: the first groups touch
            # F[0] and the center tile F[3]; param loads go next (they gate
            # the first multiply/asp), and the remaining F tiles stream in
            # while the first row's taps already run (the 7 transfers are
            # ~43us of DMA bandwidth, so ordering decides compute start).
            for oy in (0, RAD):
                make_ftile(0, oy)
            prep0 = _prep(nc, x, fpool, cpool, macros, 0)
            for oy in (1, 2, 4, 5, 6):
                make_ftile(0, oy)

            for k in range(NM):
                _macro(nc, tc, x, y, ident, fpool, cpool, dpool, spool,
                       ppool, macros, k, Fk, make_ftile,
                       prep0 if k == 0 else None)
    nc.compile()
    return nc


def _prep(nc, x, fpool, cpool, macros, k):
    """Per-macro parameter tiles: P2[c] = p_c^2 (f16, c-major) and the
    16-slot spatial log-weight table slot(bi*4+ai) = D2VALS[ai]*sx2 +
    D2VALS[bi]*sy2.  Slot order is chosen so every paired group's taps
    land in adjacent slots -> the paired asp-add is one contiguous slice.
    Param DMAs ride the ACT queue (HWDGE), independent of the sync queue."""
    hb, wck = macros[k]
    w0, r0 = wck * WC, hb * 128
    P2 = cpool.tile([128, CF * WC], FP16, tag="P2", name=f"P2_{k}")
    sxy2 = cpool.tile([128, 2 * WC], FP16, tag="sxy2", name=f"sxy2_{k}")
    for kk in range(CF + 2):
        pst = fpool.tile([128, WC], FP32, tag="pst", bufs=2,
                         name=f"pst_{k}_{kk}")
        nc.scalar.dma_start(
            out=pst[:],
            in_=x[r0 + RAD : r0 + RAD + 128, CF + kk, w0 : w0 + WC])
        dst = (P2[:, kk * WC : (kk + 1) * WC] if kk < CF
               else sxy2[:, (kk - CF) * WC : (kk - CF + 1) * WC])
        nc.scalar.activation(dst, pst[:], AF.Square)
    sx2 = sxy2[:, 0:WC]
    sy2 = sxy2[:, WC : 2 * WC]
    Asp16 = cpool.tile([128, 16 * WC], FP16, tag="Asp16", name=f"A16_{k}")
    for ai in (1, 2, 3):                       # bi = 0 row
        nc.vector.tensor_scalar_mul(
            Asp16[:, ai * WC : (ai + 1) * WC], sx2, float(D2VALS[ai]))
    for bi in (1, 2, 3):                       # ai = 0 column
        nc.vector.tensor_scalar_mul(
            Asp16[:, bi * 4 * WC : (bi * 4 + 1) * WC], sy2,
            float(D2VALS[bi]))
    for ai in (1, 2, 3):
        for bi in (1, 2, 3):
            s = bi * 4 + ai
            nc.vector.tensor_add(
                Asp16[:, s * WC : (s + 1) * WC],
                Asp16[:, bi * 4 * WC : (bi * 4 + 1) * WC],
                Asp16[:, ai * WC : (ai + 1) * WC])
    return P2, Asp16[:].rearrange("p (s x) -> p s x", s=16)


def _macro(nc, tc, x, y, ident, fpool, cpool, dpool, spool, ppool,
           macros, k, Fk, make_ftile, prep=None):
    hb, wck = macros[k]
    NM = len(macros)
    w0 = wck * WC
    r0 = hb * 128
    F = Fk[k]
    Fc = _cm(F[RAD][:])[:, :, RAD : RAD + WC]

    P2, A3 = prep if prep is not None else _prep(nc, x, fpool, cpool,
                                                macros, k)

    def asp_slot(i, j):
        return D2IDX[i] * 4 + D2IDX[j]

    # ---- PSUM accumulator: [w*fn0 | w*fn1 | w*fn2 | w] ----
    ps = ppool.tile([128, 4 * WC], FP32, tag="ps", bufs=2, name=f"ps_{k}")

    # ---- tap groups: pairs of taps share one d-tile and run the square,
    # p^2-multiply, tree, asp and exp as single double-width ops, halving
    # the ~150ns fixed cost per DVE op.  Within a pair the taps are ordered
    # by asp slot so the paired asp-add reads adjacent slots. ----
    groups = []          # (i, [j...]) with len 1 or 2; None marks center
    row_last = {}
    for i in range(KS):
        gl = ([[0, 1], [2, 3], [4, 5], [6]] if i != RAD
              else [[0, 1], [2, 4], [5, 6], None])
        for js in gl:
            if js is None:
                groups.append((i, None))
            else:
                groups.append((i, sorted(js, key=lambda j: asp_slot(i, j))))
        row_last[i] = len(groups) - 1
    n = len(groups)
    DW = 2 * CF * WC     # d-tile width (two taps)
    TW = 2 * 4 * WC
    Dt, Tt = {}, {}

    def gv(ap, G, w):    # [128, G*w] -> [128, G, w]
        return ap.rearrange("p (g x) -> p g x", g=G)

    def stage_sub(g):    # per-tap subs into the halves of one shared tile
        i, js = groups[g]
        if js is None:
            return
        d = dpool.tile([128, DW], FP16, tag="d", bufs=4, name=f"d_{k}_{g}")
        for gi, j in enumerate(js):
            nc.vector.tensor_sub(
                _cm(d[:, gi * CF * WC : (gi + 1) * CF * WC], WC),
                _cm(F[i][:])[:, :, j : j + WC], Fc)
        Dt[g] = d

    def stage_sq(g):    # one ACT square over both halves, in-place
        i, js = groups[g]
        if js is None:
            return
        d = Dt[g]
        nc.scalar.activation(d[:, 0 : len(js) * CF * WC],
                             d[:, 0 : len(js) * CF * WC], AF.Square)

    def stage_m(g):     # m = p^2 * q over both halves (P2 broadcast)
        i, js = groups[g]
        if js is None:
            return
        G = len(js)
        dv = gv(Dt[g][:, 0 : G * CF * WC], G, CF * WC)
        p2b = P2[:].unsqueeze(1).broadcast_to([128, G, CF * WC])
        nc.vector.tensor_mul(dv, p2b, dv)

    def tree(g, lvl):   # halve each tap's channel block, both taps at once

        i, js = groups[g]
        if js is None:
            return
        G = len(js)
        hw = (CF >> (lvl - 1)) * WC       # block width entering this level
        dv = gv(Dt[g][:, 0 : G * CF * WC], G, CF * WC)
        nc.vector.tensor_add(dv[:, :, 0 : hw // 2], dv[:, :, 0 : hw // 2],
                             dv[:, :, hw // 2 : hw])

    def stage_we(g):    # +asp (paired slot read), exp -> w
        i, js = groups[g]
        T = spool.tile([128, TW], FP16, tag="T", bufs=4, name=f"T_{k}_{g}")
        Tt[g] = T
        if js is None:
            nc.gpsimd.memset(T[:, 3 * WC : 4 * WC], 1.0)
            return
        G = len(js)
        dv = gv(Dt.pop(g)[:, 0 : G * CF * WC], G, CF * WC)[:, :, 0:WC]
        s0 = asp_slot(i, js[0])
        if G == 2:
            s1 = asp_slot(i, js[1])
            aspv = (A3[:, s0 : s0 + 2, :] if s1 == s0 + 1
                    else A3[:, s0 : s0 + 1, :].broadcast_to([128, 2, WC]))
        else:
            aspv = A3[:, s0 : s0 + 1, :]
        nc.vector.tensor_add(dv, dv, aspv)
        tv = gv(T[:, 0 : G * 4 * WC], G, 4 * WC)
        nc.scalar.activation(tv[:, :, 3 * WC : 4 * WC], dv, AF.Exp,
                             scale=-1.0)

    def stage_c(g):     # w*fn3 per tap, PE accumulates [w*fn3 | w] chunks
        i, js = groups[g]
        T = Tt.pop(g)
        if js is None:
            fn3 = _cm(F[i][:])[:, 0:CO, RAD : RAD + WC]
            nc.vector.tensor_copy(_cm(T[:, 0 : CO * WC], WC, CO), fn3)
            js_eff = [RAD]
        else:
            js_eff = js
            for gi, j in enumerate(js):
                o = gi * 4 * WC
                w_b = T[:, o + 3 * WC : o + 4 * WC].unsqueeze(1).broadcast_to(
                    [128, CO, WC])
                fn3 = _cm(F[i][:])[:, 0:CO, j : j + WC]
                nc.vector.tensor_mul(
                    _cm(T[:, o : o + CO * WC], WC, CO), w_b, fn3)
        for gi in range(len(js_eff)):
            for kk in range(4):
                nc.tensor.matmul(
                    ps[:, kk * WC : (kk + 1) * WC], ident[:],
                    T[:, (gi * 4 + kk) * WC : (gi * 4 + kk + 1) * WC],
                    start=(g == 0 and gi == 0),
                    stop=(g == n - 1 and gi == len(js_eff) - 1))

    # DVE issue order interleaves the dependent tree chain of group g-2 with
    # independent ops (sub of g, m of g-1, mul3 of g-3) so each op's pipe
    # DRAIN overlaps an unrelated op instead of stalling its consumer.
    # ACT order per iteration: exp(g-2) before square(g), so the exp->mul3
    # chain never queues behind the big square.
    for g in range(n + 3):
        if 0 <= g - 2 < n:
            tree(g - 2, 1)
        if g < n:
            stage_sub(g)
        if 0 <= g - 2 < n:
            tree(g - 2, 2)
        if 0 <= g - 1 < n:
            stage_m(g - 1)
        if 0 <= g - 2 < n:
            tree(g - 2, 3)
        if 0 <= g - 3 < n:
            stage_c(g - 3)
        if 0 <= g - 2 < n:
            stage_we(g - 2)
        if g < n:
            stage_sq(g)
        # Row block i of F is last read by stage_c(row_last[i]), issued at
        # iteration row_last[i]+3: rebuild it for the next macro after that.
        if k + 1 < NM and 0 <= g - 3 < n:
            i_done, js_done = groups[g - 3]
            if g - 3 == row_last[i_done]:
                make_ftile(k + 1, i_done)

    # ---- ship [acc3 | wsum] out; host divides (keeps DVE out of the tail,
    # the PSUM->SBUF copy rides the otherwise idle ACT engine) ----
    out4 = spool.tile([128, 4 * WC], FP32, tag="out4", name=f"o4_{k}")
    nc.scalar.copy(out4[:], ps[:])
    for c in range(CO + 1):
        nc.sync.dma_start(out=y[c, r0 : r0 + 128, w0 : w0 + WC],
                          in_=out4[:, c * WC : (c + 1) * WC])


def shard_inputs(input):
    """input [2,18,1024,1024] -> 8 per-core slabs [262, 18, 1024]."""
    input = np.asarray(input, dtype=np.float32)
    per_b = 4
    rows = H // per_b
    in_maps = []
    for core in range(8):
        b, q = divmod(core, per_b)
        r0 = q * rows
        slab = np.full((HIN, C_ALL, W), SENT, dtype=np.float32)
        s_lo = max(r0 - RAD, 0)
        s_hi = min(r0 + rows + RAD, H)
        slab[s_lo - (r0 - RAD) : s_hi - (r0 - RAD), :, :] = (
            input[b, :, s_lo:s_hi, :].transpose(1, 0, 2))
        in_maps.append({"x": np.ascontiguousarray(slab)})
    return in_maps


def assemble(results):
    out = np.empty((B, CO, H, W), dtype=np.float32)
    rows = H // 4
    for core in range(8):
        b, q = divmod(core, 4)
        y4 = results[core]["y"]
        out[b, :, q * rows : (q + 1) * rows, :] = y4[:CO] / y4[CO]
    return out


def kernel(input):
    from concourse.bass_utils import run_bass_kernel_spmd

    if "nc" not in _CACHED:
        _CACHED["nc"] = build_nc()
    in_maps = shard_inputs(input)
    res = run_bass_kernel_spmd(_CACHED["nc"], in_maps, list(range(8)))
    return assemble(res.results)


# revision 41
# speedup vs baseline: 1.2101x; 1.0019x over previous
"""Bilateral filter (7x7, dilation 1) Trainium2 Bass kernel — v7.

Problem: input [2, 18, 1024, 1024] f32.
  filterable = input[:, :8]; params = input[:, 8:]
  r_c = -(p_c^2), sx = -(p8^2), sy = -(p9^2)
  logw = sum_c r_c (fn_c - f_c)^2 + sx dx^2 + sy dy^2  (OOB taps masked)
  out[c] = sum_taps w * fn_c / sum_taps w,  c < 3

Sharding: data-parallel over (batch, H): 8 cores, each 256 rows of one batch
image (+3 halo rows each side, sentinel-padded host-side, sentinel=8).

Design (per core, 2 row-blocks x 2 W-chunks of [128 rows, 512 cols]):
  - fp16 on-chip compute; channel-planar free-axis layout [128, 8ch*518col]
    keeps every hot access-pattern unit-stride => DVE 2x_1P mode throughout.
  - Hot loop runs on DVE+ACT only.  GPSIMD tensor ops share the DVE SBUF
    port and throttle DVE ~1.8x while active (measured), so GPSIMD only
    does cast-DMA descriptor generation and memsets.
  - Per tap: DVE sub -> ACT Square (in-place) -> DVE m=p^2*q ->
    DVE pairwise-tree channel reduce -> +Asp -> ACT exp(scale=-1) ->
    DVE w*fn3 -> PE identity-matmul accumulates [w*fn3 | w] into PSUM
    (fp32) across all 49 taps.  4-stage software pipeline so every
    cross-engine dependency is issued >=1 full tap ahead.
  - Macro-boundary overlap: staging tiles are double-buffered and loaded
    (GPSIMD cast-DMA f32->f16) during the previous macro's taps; the 12
    partition-shifted SBUF->SBUF copies that build the 7 row-shifted tile
    sets are issued as soon as the tap loop stops reading each F tile.
  - Weights: w = exp(-(sum_c p_c^2 d_c^2 + a*sx^2 + b*sy^2)) with all
    terms positive; sentinel pixels drive the sum to ~1e4 so exp -> +0,
    reproducing the reference's OOB mask (no inf anywhere: max sum ~41K
    < fp16 max 65504).
"""

import sys

if "/opt/trn_rl_repo" not in sys.path:
    sys.path.insert(0, "/opt/trn_rl_repo")

import numpy as np

import concourse.bass as bass
import concourse.mybir as mybir
from concourse.bacc import Bacc
from concourse.tile import TileContext
from concourse.masks import make_identity

FP32 = mybir.dt.float32
FP16 = mybir.dt.float16
AF = mybir.ActivationFunctionType

B, C_ALL, H, W = 2, 18, 1024, 1024
CF = 8                      # filterable channels
CO = 3                      # output channels
KS, RAD = 7, 3
HC = H * B // 8             # 256 output rows per core
HIN = HC + 2 * RAD          # 262 input rows per core (halo padded host-side)
WC = 512                    # W chunk
NW = W // WC                # 2
NHB = HC // 128             # 2
WT = WC + 2 * RAD           # 518 (with column halo)
SENT = 8.0                  # sentinel: max quadratic form ~41K < fp16 max,
                            # so no inf on-chip, yet exp(-s) underflows to +0
D2IDX = [3, 2, 1, 0, 1, 2, 3]              # index into D2VALS: (k-3)^2
D2VALS = [0.0, 1.0, 4.0, 9.0]
CTR = KS // 2 * KS + KS // 2               # center tap (w == 1 fast path)

_CACHED = {}


def _cm(ap, w=WT, c=CF):
    """View flat [128, c*w] as [128, c, w] (channel-major blocks)."""
    return ap.rearrange("p (c x) -> p c x", c=c)


def build_nc():
    nc = Bacc()
    x = nc.dram_tensor("x", [HIN, C_ALL, W], FP32, kind="ExternalInput")
    # y carries [acc0, acc1, acc2, wsum]; the division happens on the host
    # (DVE reciprocal is an iterative ~8cyc/elem op — not worth DVE time)
    y = nc.dram_tensor("y", [CO + 1, HC, W], FP32, kind="ExternalOutput")

    macros = [(hb, wck) for hb in range(NHB) for wck in range(NW)]
    NM = len(macros)

    with TileContext(nc) as tc:
        with (
            tc.tile_pool(name="ipool", bufs=1) as ipool,
            tc.tile_pool(name="fpool", bufs=1) as fpool,
            tc.tile_pool(name="cpool", bufs=1) as cpool,
            tc.tile_pool(name="dpool", bufs=1) as dpool,
            tc.tile_pool(name="spool", bufs=1) as spool,
            tc.tile_pool(name="ppool", bufs=1, space="PSUM") as ppool,
        ):
            ident = ipool.tile([128, 128], FP16, tag="ident", name="ident")
            make_identity(nc, ident[:])

            Fk = {}     # macro idx -> {oy: tile}

            def make_ftile(k, oy):
                """F[oy][p] = slab row r0+oy+p, cast f32->f16 straight from
                DRAM (write-only SBUF traffic: no staging, no SBUF->SBUF
                copies contending with DVE reads)."""
                hb, wck = macros[k]
                w0, r0 = wck * WC, hb * 128
                lo = RAD if wck == 0 else 0
                hi = WT - RAD if wck == NW - 1 else WT
                Ft = fpool.tile([128, CF * WT], FP16, tag=f"F{oy}",
                                bufs=2 if oy == RAD else 1,
                                name=f"F{oy}_{k}")
                v = Ft[:].rearrange("p (c x) -> p c x", c=CF)
                if lo > 0:
                    nc.gpsimd.memset(v[:, :, 0:lo], SENT)
                if hi < WT:
                    nc.gpsimd.memset(v[:, :, hi:WT], SENT)
                nc.gpsimd.dma_start(
                    out=v[:, :, lo:hi],
                    in_=x[r0 + oy : r0 + oy + 128, 0:CF,
                          w0 - RAD + lo : w0 - RAD + hi],
                )
                Fk.setdefault(k, {})[oy] = Ft

            # Load order = tap consumption order: the first groups touch
            # F[0] and the center tile F[3]; param loads go next (they gate
            # the first multiply/asp), and the remaining F tiles stream in
            # while the first row's taps already run (the 7 transfers are
            # ~43us of DMA bandwidth, so ordering decides compute start).
            for oy in (0, RAD):
                make_ftile(0, oy)
            prep0 = _prep(nc, x, fpool, cpool, macros, 0)
            for oy in (1, 2, 4, 5, 6):
                make_ftile(0, oy)

            for k in range(NM):
                _macro(nc, tc, x, y, ident, fpool, cpool, dpool, spool,
                       ppool, macros, k, Fk, make_ftile,
                       prep0 if k == 0 else None)
    nc.compile()
    return nc


def _prep(nc, x, fpool, cpool, macros, k):
    """Per-macro parameter tiles: P2[c] = p_c^2 (f16, c-major) and the
    16-slot spatial log-weight table slot(bi*4+ai) = D2VALS[ai]*sx2 +
    D2VALS[bi]*sy2.  Slot order is chosen so every paired group's taps
    land in adjacent slots -> the paired asp-add is one contiguous slice.
    Param DMAs ride the ACT queue (HWDGE), independent of the sync queue."""
    hb, wck = macros[k]
    w0, r0 = wck * WC, hb * 128
    P2 = cpool.tile([128, CF * WC], FP16, tag="P2", name=f"P2_{k}")
    sxy2 = cpool.tile([128, 2 * WC], FP16, tag="sxy2", name=f"sxy2_{k}")
    for kk in range(CF + 2):
        pst = fpool.tile([128, WC], FP32, tag="pst", bufs=2,
                         name=f"pst_{k}_{kk}")
        nc.scalar.dma_start(
            out=pst[:],
            in_=x[r0 + RAD : r0 + RAD + 128, CF + kk, w0 : w0 + WC])
        dst = (P2[:, kk * WC : (kk + 1) * WC] if kk < CF
               else sxy2[:, (kk - CF) * WC : (kk - CF + 1) * WC])
        nc.scalar.activation(dst, pst[:], AF.Square)
    sx2 = sxy2[:, 0:WC]
    sy2 = sxy2[:, WC : 2 * WC]
    Asp16 = cpool.tile([128, 16 * WC], FP16, tag="Asp16", name=f"A16_{k}")
    for ai in (1, 2, 3):                       # bi = 0 row
        nc.vector.tensor_scalar_mul(
            Asp16[:, ai * WC : (ai + 1) * WC], sx2, float(D2VALS[ai]))
    for bi in (1, 2, 3):                       # ai = 0 column
        nc.vector.tensor_scalar_mul(
            Asp16[:, bi * 4 * WC : (bi * 4 + 1) * WC], sy2,
            float(D2VALS[bi]))
    A3v = Asp16[:].rearrange("p (s x) -> p s x", s=16)
    for bi in (1, 2, 3):   # slots bi*4+1..3 = bi*sy2 (bcast) + {1,4,9}*sx2
        nc.vector.tensor_add(
            A3v[:, bi * 4 + 1 : bi * 4 + 4, :],
            A3v[:, bi * 4 : bi * 4 + 1, :].broadcast_to([128, 3, WC]),
            A3v[:, 1:4, :])
    return P2, A3v


def _macro(nc, tc, x, y, ident, fpool, cpool, dpool, spool, ppool,
           macros, k, Fk, make_ftile, prep=None):
    hb, wck = macros[k]
    NM = len(macros)
    w0 = wck * WC
    r0 = hb * 128
    F = Fk[k]
    Fc = _cm(F[RAD][:])[:, :, RAD : RAD + WC]

    P2, A3 = prep if prep is not None else _prep(nc, x, fpool, cpool,
                                                macros, k)

    def asp_slot(i, j):
        return D2IDX[i] * 4 + D2IDX[j]

    # ---- PSUM accumulator: [w*fn0 | w*fn1 | w*fn2 | w] ----
    ps = ppool.tile([128, 4 * WC], FP32, tag="ps", bufs=2, name=f"ps_{k}")

    # ---- tap groups: pairs of taps share one d-tile and run the square,
    # p^2-multiply, tree, asp and exp as single double-width ops, halving
    # the ~150ns fixed cost per DVE op.  Within a pair the taps are ordered
    # by asp slot so the paired asp-add reads adjacent slots. ----
    groups = []          # (i, [j...]) with len 1 or 2; None marks center
    row_last = {}
    for i in range(KS):
        gl = ([[0, 1], [2, 3], [4, 5], [6]] if i != RAD
              else [[0, 1], [2, 4], [5, 6], None])
        for js in gl:
            if js is None:
                groups.append((i, None))
            else:
                groups.append((i, sorted(js, key=lambda j: asp_slot(i, j))))
        row_last[i] = len(groups) - 1
    n = len(groups)
    DW = 2 * CF * WC     # d-tile width (two taps)
    TW = 2 * 4 * WC
    Dt, Tt = {}, {}

    def gv(ap, G, w):    # [128, G*w] -> [128, G, w]
        return ap.rearrange("p (g x) -> p g x", g=G)

    def stage_sub(g):    # per-tap subs into the halves of one shared tile
        i, js = groups[g]
        if js is None:
            return
        d = dpool.tile([128, DW], FP16, tag="d", bufs=4, name=f"d_{k}_{g}")
        for gi, j in enumerate(js):
            nc.vector.tensor_sub(
                _cm(d[:, gi * CF * WC : (gi + 1) * CF * WC], WC),
                _cm(F[i][:])[:, :, j : j + WC], Fc)
        Dt[g] = d

    def stage_sq(g):    # one ACT square over both halves, in-place
        i, js = groups[g]
        if js is None:
            return
        d = Dt[g]
        nc.scalar.activation(d[:, 0 : len(js) * CF * WC],
                             d[:, 0 : len(js) * CF * WC], AF.Square)

    def stage_m(g):     # m = p^2 * q over both halves (P2 broadcast)
        i, js = groups[g]
        if js is None:
            return
        G = len(js)
        dv = gv(Dt[g][:, 0 : G * CF * WC], G, CF * WC)
        p2b = P2[:].unsqueeze(1).broadcast_to([128, G, CF * WC])
        nc.vector.tensor_mul(dv, p2b, dv)

    def tree(g, lvl):   # halve each tap's channel block, both taps at once

        i, js = groups[g]
        if js is None:
            return
        G = len(js)
        hw = (CF >> (lvl - 1)) * WC       # block width entering this level
        dv = gv(Dt[g][:, 0 : G * CF * WC], G, CF * WC)
        nc.vector.tensor_add(dv[:, :, 0 : hw // 2], dv[:, :, 0 : hw // 2],
                             dv[:, :, hw // 2 : hw])

    def stage_we(g):    # +asp (paired slot read), exp -> w
        i, js = groups[g]
        T = spool.tile([128, TW], FP16, tag="T", bufs=4, name=f"T_{k}_{g}")
        Tt[g] = T
        if js is None:
            nc.gpsimd.memset(T[:, 3 * WC : 4 * WC], 1.0)
            return
        G = len(js)
        dv = gv(Dt.pop(g)[:, 0 : G * CF * WC], G, CF * WC)[:, :, 0:WC]
        s0 = asp_slot(i, js[0])
        if G == 2:
            s1 = asp_slot(i, js[1])
            aspv = (A3[:, s0 : s0 + 2, :] if s1 == s0 + 1
                    else A3[:, s0 : s0 + 1, :].broadcast_to([128, 2, WC]))
        else:
            aspv = A3[:, s0 : s0 + 1, :]
        nc.vector.tensor_add(dv, dv, aspv)
        tv = gv(T[:, 0 : G * 4 * WC], G, 4 * WC)
        nc.scalar.activation(tv[:, :, 3 * WC : 4 * WC], dv, AF.Exp,
                             scale=-1.0)

    def stage_c(g):     # w*fn3 per tap, PE accumulates [w*fn3 | w] chunks
        i, js = groups[g]
        T = Tt.pop(g)
        if js is None:
            fn3 = _cm(F[i][:])[:, 0:CO, RAD : RAD + WC]
            nc.vector.tensor_copy(_cm(T[:, 0 : CO * WC], WC, CO), fn3)
            js_eff = [RAD]
        else:
            js_eff = js
            for gi, j in enumerate(js):
                o = gi * 4 * WC
                w_b = T[:, o + 3 * WC : o + 4 * WC].unsqueeze(1).broadcast_to(
                    [128, CO, WC])
                fn3 = _cm(F[i][:])[:, 0:CO, j : j + WC]
                nc.vector.tensor_mul(
                    _cm(T[:, o : o + CO * WC], WC, CO), w_b, fn3)
        for gi in range(len(js_eff)):
            for kk in range(4):
                nc.tensor.matmul(
                    ps[:, kk * WC : (kk + 1) * WC], ident[:],
                    T[:, (gi * 4 + kk) * WC : (gi * 4 + kk + 1) * WC],
                    start=(g == 0 and gi == 0),
                    stop=(g == n - 1 and gi == len(js_eff) - 1))

    # DVE issue order interleaves the dependent tree chain of group g-2 with
    # independent ops (sub of g, m of g-1, mul3 of g-3) so each op's pipe
    # DRAIN overlaps an unrelated op instead of stalling its consumer.
    # ACT order per iteration: exp(g-2) before square(g), so the exp->mul3
    # chain never queues behind the big square.
    for g in range(n + 3):
        if 0 <= g - 2 < n:
            tree(g - 2, 1)
        if g < n:
            stage_sub(g)
        if 0 <= g - 2 < n:
            tree(g - 2, 2)
        if 0 <= g - 1 < n:
            stage_m(g - 1)
        if 0 <= g - 2 < n:
            tree(g - 2, 3)
        if 0 <= g - 3 < n:
            stage_c(g - 3)
        if 0 <= g - 2 < n:
            stage_we(g - 2)
        if g < n:
            stage_sq(g)
        # Row block i of F is last read by stage_c(row_last[i]), issued at
        # iteration row_last[i]+3: rebuild it for the next macro after that.
        if k + 1 < NM and 0 <= g - 3 < n:
            i_done, js_done = groups[g - 3]
            if g - 3 == row_last[i_done]:
                make_ftile(k + 1, i_done)

    # ---- ship [acc3 | wsum] out; host divides (keeps DVE out of the tail,
    # the PSUM->SBUF copy rides the otherwise idle ACT engine) ----
    out4 = spool.tile([128, 4 * WC], FP32, tag="out4", name=f"o4_{k}")
    nc.scalar.copy(out4[:], ps[:])
    for c in range(CO + 1):
        nc.sync.dma_start(out=y[c, r0 : r0 + 128, w0 : w0 + WC],
                          in_=out4[:, c * WC : (c + 1) * WC])


def shard_inputs(input):
    """input [2,18,1024,1024] -> 8 per-core slabs [262, 18, 1024]."""
    input = np.asarray(input, dtype=np.float32)
    per_b = 4
    rows = H // per_b
    in_maps = []
    for core in range(8):
        b, q = divmod(core, per_b)
        r0 = q * rows
        slab = np.full((HIN, C_ALL, W), SENT, dtype=np.float32)
        s_lo = max(r0 - RAD, 0)
        s_hi = min(r0 + rows + RAD, H)
        slab[s_lo - (r0 - RAD) : s_hi - (r0 - RAD), :, :] = (
            input[b, :, s_lo:s_hi, :].transpose(1, 0, 2))
        in_maps.append({"x": np.ascontiguousarray(slab)})
    return in_maps


def assemble(results):
    out = np.empty((B, CO, H, W), dtype=np.float32)
    rows = H // 4
    for core in range(8):
        b, q = divmod(core, 4)
        y4 = results[core]["y"]
        out[b, :, q * rows : (q + 1) * rows, :] = y4[:CO] / y4[CO]
    return out


def kernel(input):
    from concourse.bass_utils import run_bass_kernel_spmd

    if "nc" not in _CACHED:
        _CACHED["nc"] = build_nc()
    in_maps = shard_inputs(input)
    res = run_bass_kernel_spmd(_CACHED["nc"], in_maps, list(range(8)))
    return assemble(res.results)


# revision 49
# speedup vs baseline: 1.2126x; 1.0021x over previous
"""Bilateral filter (7x7, dilation 1) Trainium2 Bass kernel — v7.

Problem: input [2, 18, 1024, 1024] f32.
  filterable = input[:, :8]; params = input[:, 8:]
  r_c = -(p_c^2), sx = -(p8^2), sy = -(p9^2)
  logw = sum_c r_c (fn_c - f_c)^2 + sx dx^2 + sy dy^2  (OOB taps masked)
  out[c] = sum_taps w * fn_c / sum_taps w,  c < 3

Sharding: data-parallel over (batch, H): 8 cores, each 256 rows of one batch
image (+3 halo rows each side, sentinel-padded host-side, sentinel=8).

Design (per core, 2 row-blocks x 2 W-chunks of [128 rows, 512 cols]):
  - fp16 on-chip compute; channel-planar free-axis layout [128, 8ch*518col]
    keeps every hot access-pattern unit-stride => DVE 2x_1P mode throughout.
  - Hot loop runs on DVE+ACT only.  GPSIMD tensor ops share the DVE SBUF
    port and throttle DVE ~1.8x while active (measured), so GPSIMD only
    does cast-DMA descriptor generation and memsets.
  - Per tap: DVE sub -> ACT Square (in-place) -> DVE m=p^2*q ->
    DVE pairwise-tree channel reduce -> +Asp -> ACT exp(scale=-1) ->
    DVE w*fn3 -> PE identity-matmul accumulates [w*fn3 | w] into PSUM
    (fp32) across all 49 taps.  4-stage software pipeline so every
    cross-engine dependency is issued >=1 full tap ahead.
  - Macro-boundary overlap: staging tiles are double-buffered and loaded
    (GPSIMD cast-DMA f32->f16) during the previous macro's taps; the 12
    partition-shifted SBUF->SBUF copies that build the 7 row-shifted tile
    sets are issued as soon as the tap loop stops reading each F tile.
  - Weights: w = exp(-(sum_c p_c^2 d_c^2 + a*sx^2 + b*sy^2)) with all
    terms positive; sentinel pixels drive the sum to ~1e4 so exp -> +0,
    reproducing the reference's OOB mask (no inf anywhere: max sum ~41K
    < fp16 max 65504).
"""

import sys

if "/opt/trn_rl_repo" not in sys.path:
    sys.path.insert(0, "/opt/trn_rl_repo")

import numpy as np

import concourse.bass as bass
import concourse.mybir as mybir
from concourse.bacc import Bacc
from concourse.tile import TileContext
from concourse.masks import make_identity

FP32 = mybir.dt.float32
FP16 = mybir.dt.float16
AF = mybir.ActivationFunctionType

B, C_ALL, H, W = 2, 18, 1024, 1024
CF = 8                      # filterable channels
CO = 3                      # output channels
KS, RAD = 7, 3
HC = H * B // 8             # 256 output rows per core
HIN = HC + 2 * RAD          # 262 input rows per core (halo padded host-side)
WC = 512                    # W chunk
NW = W // WC                # 2
NHB = HC // 128             # 2
WT = WC + 2 * RAD           # 518 (with column halo)
SENT = 8.0                  # sentinel: max quadratic form ~41K < fp16 max,
                            # so no inf on-chip, yet exp(-s) underflows to +0
D2IDX = [3, 2, 1, 0, 1, 2, 3]              # index into D2VALS: (k-3)^2
D2VALS = [0.0, 1.0, 4.0, 9.0]
CTR = KS // 2 * KS + KS // 2               # center tap (w == 1 fast path)

_CACHED = {}


def _cm(ap, w=WT, c=CF):
    """View flat [128, c*w] as [128, c, w] (channel-major blocks)."""
    return ap.rearrange("p (c x) -> p c x", c=c)


def build_nc():
    nc = Bacc()
    x = nc.dram_tensor("x", [HIN, C_ALL, W], FP32, kind="ExternalInput")
    # y carries [acc0, acc1, acc2, wsum]; the division happens on the host
    # (DVE reciprocal is an iterative ~8cyc/elem op — not worth DVE time)
    y = nc.dram_tensor("y", [CO + 1, HC, W], FP32, kind="ExternalOutput")

    macros = [(hb, wck) for hb in range(NHB) for wck in range(NW)]
    NM = len(macros)

    with TileContext(nc) as tc:
        with (
            tc.tile_pool(name="ipool", bufs=1) as ipool,
            tc.tile_pool(name="fpool", bufs=1) as fpool,
            tc.tile_pool(name="cpool", bufs=1) as cpool,
            tc.tile_pool(name="dpool", bufs=1) as dpool,
            tc.tile_pool(name="spool", bufs=1) as spool,
            tc.tile_pool(name="ppool", bufs=1, space="PSUM") as ppool,
        ):
            ident = ipool.tile([128, 128], FP16, tag="ident", name="ident")
            make_identity(nc, ident[:])

            Fk = {}     # macro idx -> {oy: tile}

            def make_ftile(k, oy):
                """F[oy][p] = slab row r0+oy+p, cast f32->f16 straight from
                DRAM (write-only SBUF traffic: no staging, no SBUF->SBUF
                copies contending with DVE reads)."""
                hb, wck = macros[k]
                w0, r0 = wck * WC, hb * 128
                lo = RAD if wck == 0 else 0
                hi = WT - RAD if wck == NW - 1 else WT
                Ft = fpool.tile([128, CF * WT], FP16, tag=f"F{oy}",
                                bufs=2 if oy == RAD else 1,
                                name=f"F{oy}_{k}")
                v = Ft[:].rearrange("p (c x) -> p c x", c=CF)
                if lo > 0:
                    nc.gpsimd.memset(v[:, :, 0:lo], SENT)
                if hi < WT:
                    nc.gpsimd.memset(v[:, :, hi:WT], SENT)
                nc.gpsimd.dma_start(
                    out=v[:, :, lo:hi],
                    in_=x[r0 + oy : r0 + oy + 128, 0:CF,
                          w0 - RAD + lo : w0 - RAD + hi],
                )
                Fk.setdefault(k, {})[oy] = Ft

            # Load order = tap consumption order: the first groups touch
            # F[0] and the center tile F[3]; param loads go next (they gate
            # the first multiply/asp), and the remaining F tiles stream in
            # while the first row's taps already run (the 7 transfers are
            # ~43us of DMA bandwidth, so ordering decides compute start).
            for oy in (0, RAD):
                make_ftile(0, oy)
            prep0 = _prep(nc, x, fpool, cpool, macros, 0)
            for oy in (1, 2, 4, 5, 6):
                make_ftile(0, oy)

            for k in range(NM):
                _macro(nc, tc, x, y, ident, fpool, cpool, dpool, spool,
                       ppool, macros, k, Fk, make_ftile,
                       prep0 if k == 0 else None)
    nc.compile()
    return nc


def _prep(nc, x, fpool, cpool, macros, k):
    """Per-macro parameter tiles: P2[c] = p_c^2 (f16, c-major) and the
    16-slot spatial log-weight table slot(bi*4+ai) = D2VALS[ai]*sx2 +
    D2VALS[bi]*sy2.  Slot order is chosen so every paired group's taps
    land in adjacent slots -> the paired asp-add is one contiguous slice.
    Param DMAs ride the ACT queue (HWDGE), independent of the sync queue."""
    hb, wck = macros[k]
    w0, r0 = wck * WC, hb * 128
    P2 = cpool.tile([128, CF * WC], FP16, tag="P2", name=f"P2_{k}")
    sxy2 = cpool.tile([128, 2 * WC], FP16, tag="sxy2", name=f"sxy2_{k}")
    for kk in range(CF + 2):
        pst = fpool.tile([128, WC], FP32, tag="pst", bufs=2,
                         name=f"pst_{k}_{kk}")
        nc.scalar.dma_start(
            out=pst[:],
            in_=x[r0 + RAD : r0 + RAD + 128, CF + kk, w0 : w0 + WC])
        dst = (P2[:, kk * WC : (kk + 1) * WC] if kk < CF
               else sxy2[:, (kk - CF) * WC : (kk - CF + 1) * WC])
        nc.scalar.activation(dst, pst[:], AF.Square)
    sx2 = sxy2[:, 0:WC]
    sy2 = sxy2[:, WC : 2 * WC]
    Asp16 = cpool.tile([128, 16 * WC], FP16, tag="Asp16", name=f"A16_{k}")
    for ai in (1, 2, 3):                       # bi = 0 row
        nc.vector.tensor_scalar_mul(
            Asp16[:, ai * WC : (ai + 1) * WC], sx2, float(D2VALS[ai]))
    for bi in (1, 2, 3):                       # ai = 0 column
        nc.vector.tensor_scalar_mul(
            Asp16[:, bi * 4 * WC : (bi * 4 + 1) * WC], sy2,
            float(D2VALS[bi]))
    A3v = Asp16[:].rearrange("p (s x) -> p s x", s=16)
    for bi in (1, 2, 3):   # slots bi*4+1..3 = bi*sy2 (bcast) + {1,4,9}*sx2
        nc.vector.tensor_add(
            A3v[:, bi * 4 + 1 : bi * 4 + 4, :],
            A3v[:, bi * 4 : bi * 4 + 1, :].broadcast_to([128, 3, WC]),
            A3v[:, 1:4, :])
    # second view of the same table at stride 4 (slot g*4+3 = 9sx2+g*sy2
    # lives at column offset 3*WC of group g) for cross-row j=6 pairs
    return P2, A3v, Asp16[:].rearrange("p (s x) -> p s x", s=4)


def _macro(nc, tc, x, y, ident, fpool, cpool, dpool, spool, ppool,
           macros, k, Fk, make_ftile, prep=None):
    hb, wck = macros[k]
    NM = len(macros)
    w0 = wck * WC
    r0 = hb * 128
    F = Fk[k]
    Fc = _cm(F[RAD][:])[:, :, RAD : RAD + WC]

    P2, A3, A3v4 = prep if prep is not None else _prep(nc, x, fpool, cpool,
                                                       macros, k)

    def asp_slot(i, j):
        return D2IDX[i] * 4 + D2IDX[j]

    # ---- PSUM accumulator: [w*fn0 | w*fn1 | w*fn2 | w] ----
    ps = ppool.tile([128, 4 * WC], FP32, tag="ps", bufs=2, name=f"ps_{k}")

    # ---- tap groups: pairs of taps share one d-tile and run the square,
    # p^2-multiply, tree, asp and exp as single double-width ops, halving
    # the ~150ns fixed cost per DVE op.  Within a pair the taps are ordered
    # by asp slot so the paired asp-add reads adjacent slots. ----
    skey = lambda t: asp_slot(*t)
    groups = []          # list[(i,j)] pairs; None marks the center tap
    for i in range(KS):
        gl = ([[(i, 0), (i, 1)], [(i, 2), (i, 3)], [(i, 4), (i, 5)]]
              if i != RAD else
              [[(i, 0), (i, 1)], [(i, 2), (i, 4)], [(i, 5), (i, 6)], None])
        for ts in gl:
            groups.append(sorted(ts, key=skey) if ts else None)
        # the j=6 leftovers pair ACROSS rows (their asp values live at
        # slot bi*4+3, reachable as a stride-4 view of Asp16); emit each
        # cross-pair once its later row's tiles are loaded
        if i == 1:
            groups.append(sorted([(0, 6), (1, 6)], key=skey))
        elif i == 4:
            groups.append([(2, 6), (4, 6)])     # equal slots
        elif i == 6:
            groups.append(sorted([(5, 6), (6, 6)], key=skey))
    n = len(groups)
    row_last = {}
    for gi, ts in enumerate(groups):
        for i in ({t[0] for t in ts} if ts else {RAD}):
            row_last[i] = gi
    DW = 2 * CF * WC     # d-tile width (two taps)
    TW = 2 * 4 * WC
    Dt, Tt = {}, {}

    def gv(ap, G, w):    # [128, G*w] -> [128, G, w]
        return ap.rearrange("p (g x) -> p g x", g=G)

    def stage_sub(g):    # per-tap subs into the halves of one shared tile
        ts = groups[g]
        if ts is None:
            return
        d = dpool.tile([128, DW], FP16, tag="d", bufs=4, name=f"d_{k}_{g}")
        for gi, (ti, tj) in enumerate(ts):
            nc.vector.tensor_sub(
                _cm(d[:, gi * CF * WC : (gi + 1) * CF * WC], WC),
                _cm(F[ti][:])[:, :, tj : tj + WC], Fc)
        Dt[g] = d

    def stage_sq(g):    # one ACT square over both halves, in-place
        ts = groups[g]
        if ts is None:
            return
        d = Dt[g]
        nc.scalar.activation(d[:, 0 : len(ts) * CF * WC],
                             d[:, 0 : len(ts) * CF * WC], AF.Square)

    def stage_m(g):     # m = p^2 * q over both halves (P2 broadcast)
        ts = groups[g]
        if ts is None:
            return
        G = len(ts)
        dv = gv(Dt[g][:, 0 : G * CF * WC], G, CF * WC)
        p2b = P2[:].unsqueeze(1).broadcast_to([128, G, CF * WC])
        nc.vector.tensor_mul(dv, p2b, dv)

    def tree(g, lvl):   # halve each tap's channel block, both taps at once
        ts = groups[g]
        if ts is None:
            return
        G = len(ts)
        hw = (CF >> (lvl - 1)) * WC       # block width entering this level
        dv = gv(Dt[g][:, 0 : G * CF * WC], G, CF * WC)
        nc.vector.tensor_add(dv[:, :, 0 : hw // 2], dv[:, :, 0 : hw // 2],
                             dv[:, :, hw // 2 : hw])

    def stage_we(g):    # +asp (grouped slot read), exp -> w
        ts = groups[g]
        T = spool.tile([128, TW], FP16, tag="T", bufs=4, name=f"T_{k}_{g}")
        Tt[g] = T
        if ts is None:
            nc.gpsimd.memset(T[:, 3 * WC : 4 * WC], 1.0)
            return
        G = len(ts)
        dv = gv(Dt.pop(g)[:, 0 : G * CF * WC], G, CF * WC)[:, :, 0:WC]
        if all(t[0] == ts[0][0] for t in ts):   # same-row: adjacent slots
            s0 = asp_slot(*ts[0])
            if G == 2 and asp_slot(*ts[1]) != s0 + 1:
                aspv = A3[:, s0 : s0 + 1, :].broadcast_to([128, 2, WC])
            else:
                aspv = A3[:, s0 : s0 + G, :]
        else:       # cross-row j=6 pair: slots bi*4+3 via the stride-4 view
            b0, b1 = (D2IDX[t[0]] for t in ts)
            if b1 == b0 + 1:
                aspv = A3v4[:, b0 : b0 + 2, 3 * WC : 4 * WC]
            else:
                aspv = A3v4[:, b0 : b0 + 1,
                            3 * WC : 4 * WC].broadcast_to([128, 2, WC])
        nc.vector.tensor_add(dv, dv, aspv)
        tv = gv(T[:, 0 : G * 4 * WC], G, 4 * WC)
        nc.scalar.activation(tv[:, :, 3 * WC : 4 * WC], dv, AF.Exp,
                             scale=-1.0)

    def stage_c(g):     # w*fn3 per tap, PE accumulates [w*fn3 | w] chunks
        ts = groups[g]
        T = Tt.pop(g)
        if ts is None:
            fn3 = _cm(F[RAD][:])[:, 0:CO, RAD : RAD + WC]
            nc.vector.tensor_copy(_cm(T[:, 0 : CO * WC], WC, CO), fn3)
            ts_eff = [(RAD, RAD)]
        else:
            ts_eff = ts
            for gi, (ti, tj) in enumerate(ts):
                o = gi * 4 * WC
                w_b = T[:, o + 3 * WC : o + 4 * WC].unsqueeze(1).broadcast_to(
                    [128, CO, WC])
                fn3 = _cm(F[ti][:])[:, 0:CO, tj : tj + WC]
                nc.vector.tensor_mul(
                    _cm(T[:, o : o + CO * WC], WC, CO), w_b, fn3)
        for gi in range(len(ts_eff)):
            for kk in range(4):
                nc.tensor.matmul(
                    ps[:, kk * WC : (kk + 1) * WC], ident[:],
                    T[:, (gi * 4 + kk) * WC : (gi * 4 + kk + 1) * WC],
                    start=(g == 0 and gi == 0),
                    stop=(g == n - 1 and gi == len(ts_eff) - 1))

    # DVE issue order interleaves the dependent tree chain of group g-2 with
    # independent ops (sub of g, m of g-1, mul3 of g-3) so each op's pipe
    # DRAIN overlaps an unrelated op instead of stalling its consumer.
    # ACT order per iteration: exp(g-2) before square(g), so the exp->mul3
    # chain never queues behind the big square.
    for g in range(n + 3):
        if 0 <= g - 2 < n:
            tree(g - 2, 1)
        if g < n:
            stage_sub(g)
        if 0 <= g - 2 < n:
            tree(g - 2, 2)
        if 0 <= g - 1 < n:
            stage_m(g - 1)
        if 0 <= g - 2 < n:
            tree(g - 2, 3)
        if 0 <= g - 3 < n:
            stage_c(g - 3)
        if 0 <= g - 2 < n:
            stage_we(g - 2)
        if g < n:
            stage_sq(g)
        # Row block i of F is last read by stage_c(row_last[i]), issued at
        # iteration row_last[i]+3: rebuild it for the next macro after that.
        if k + 1 < NM and 0 <= g - 3 < n:
            ts_done = groups[g - 3]
            for i_done in ({t[0] for t in ts_done} if ts_done else {RAD}):
                if g - 3 == row_last[i_done]:
                    make_ftile(k + 1, i_done)

    # ---- ship [acc3 | wsum] out; host divides (keeps DVE out of the tail,
    # the PSUM->SBUF copy rides the otherwise idle ACT engine) ----
    out4 = spool.tile([128, 4 * WC], FP32, tag="out4", name=f"o4_{k}")
    nc.scalar.copy(out4[:], ps[:])
    for c in range(CO + 1):
        nc.sync.dma_start(out=y[c, r0 : r0 + 128, w0 : w0 + WC],
                          in_=out4[:, c * WC : (c + 1) * WC])


def shard_inputs(input):
    """input [2,18,1024,1024] -> 8 per-core slabs [262, 18, 1024]."""
    input = np.asarray(input, dtype=np.float32)
    per_b = 4
    rows = H // per_b
    in_maps = []
    for core in range(8):
        b, q = divmod(core, per_b)
        r0 = q * rows
        slab = np.full((HIN, C_ALL, W), SENT, dtype=np.float32)
        s_lo = max(r0 - RAD, 0)
        s_hi = min(r0 + rows + RAD, H)
        slab[s_lo - (r0 - RAD) : s_hi - (r0 - RAD), :, :] = (
            input[b, :, s_lo:s_hi, :].transpose(1, 0, 2))
        in_maps.append({"x": np.ascontiguousarray(slab)})
    return in_maps


def assemble(results):
    out = np.empty((B, CO, H, W), dtype=np.float32)
    rows = H // 4
    for core in range(8):
        b, q = divmod(core, 4)
        y4 = results[core]["y"]
        out[b, :, q * rows : (q + 1) * rows, :] = y4[:CO] / y4[CO]
    return out


def kernel(input):
    from concourse.bass_utils import run_bass_kernel_spmd

    if "nc" not in _CACHED:
        _CACHED["nc"] = build_nc()
    in_maps = shard_inputs(input)
    res = run_bass_kernel_spmd(_CACHED["nc"], in_maps, list(range(8)))
    return assemble(res.results)
